# revision 49
# baseline (speedup 1.0000x reference)
"""Trainium2 Bass kernel for nn_AttentionRelative (Swin-style relative-position-bias MHA).

Full-problem shapes: x [32, 1024, 512], HEADS=8, DIM_HEAD=64.
Sharding: data-parallel over batch across 8 NeuronCores (4 batches/core);
weights and the (host-gathered, exp'd) bias table replicated.

Device algorithm per (batch, head), all matmul inputs bf16:
  - qkv projection from host-pre-transposed xT, producing qT/kT in
    [head_dim on partitions, n free] layout (one tile per head-pair,
    allocated lazily) and V in [n on partitions, (head, head_dim) free]
    layout.
  - S^T = K @ Q^T (keys m on partitions, queries q free) -> PSUM.
  - P^T = exp(S^T) * exp(biasT)  -- ACT does exp straight out of PSUM,
    the multiply by the host-precomputed exp(bias) runs on DVE (bf16 2x)
    for six of eight chunks and on GPSIMD for two (engine balance).
  - PV is computed TRANSPOSED: out[q, d] accumulated per q-chunk with
    lhsT = P^T chunk (stationary), rhs = V chunk -- free size 64 per
    matmul instead of 512, plus per-q-chunk l columns via a ones vector
    (softmax denominator lands per-partition, no DRAM broadcast bounce).
    PSUM bank-clear semantics: one accumulation group per bank (start on
    the first matmul touching the bank, stop on the last).
  - The PV+l emission lags the S/exp/mult chain by PV_LAG mc-slots
    (global software pipeline) so the in-order PE stream never stalls
    waiting for the ACT->DVE/GPSIMD latency chain.
  - epilogue (emitted when a head's last PV pops off the lag queue):
    fast reciprocal of the l columns, then 8 per-partition tensor_scalar
    multiplies evacuate+normalize PV psum into attnU [q, qc, dpair]
    bf16; after both heads of a pair, one DMA-transpose turns attnU
    into the attn2 [dpair, qc, q] lhsT layout the output projection
    needs (14ns per 32x32 tile on the DMA engines).
  - out-proj accumulates over inner chunks; b_out is added during the
    psum evacuation (tensor_tensor add with a broadcast bias tile), and
    the per-batch result is stored with a single DMA in bf16.
  - batches are processed in pairs sharing one bias-table load per head
    (one large DMA per (pair, head), prefetched a head ahead); the next
    pair's projections are emitted in small pieces at head boundaries so
    the scheduler places them into the attention phase's engine bubbles.
"""

import numpy as np
import ml_dtypes

B_FULL = 32
N_CORES = 8
B_LOC = B_FULL // N_CORES  # 4
N = 1024
D = 512
HEADS = 8
DH = 64
NCHUNK = N // 128  # 8
DCHUNK = D // 128  # 4
PV_LAG = 6  # mc-slots of lag between the S/exp/mult chain and PV emission
POOL_MC = (0, 4, 6)  # mc chunks whose exp(bias) multiply runs on GPSIMD

_PROG = None  # (nc, out_name) built once per process


def _build_program():
    import concourse.mybir as mybir
    import concourse.tile as tile
    from concourse import bacc

    f32 = mybir.dt.float32
    bf16 = mybir.dt.bfloat16
    AF = mybir.ActivationFunctionType
    OP = mybir.AluOpType

    nc = bacc.Bacc(target_bir_lowering=False)

    xt = nc.dram_tensor("xt", [B_LOC, D, N], bf16, kind="ExternalInput")
    wq = nc.dram_tensor("wq", [D, D], bf16, kind="ExternalInput")
    wk = nc.dram_tensor("wk", [D, D], bf16, kind="ExternalInput")
    wv = nc.dram_tensor("wv", [D, D], bf16, kind="ExternalInput")
    wo = nc.dram_tensor("wo", [D, D], bf16, kind="ExternalInput")
    bout = nc.dram_tensor("bout", [1, D], f32, kind="ExternalInput")
    eb = nc.dram_tensor("eb", [HEADS, N, N], bf16, kind="ExternalInput")
    out = nc.dram_tensor("out", [B_LOC, N, D], bf16, kind="ExternalOutput")

    xt_t = xt.rearrange("b (c p) n -> b p c n", p=128)      # [B, 128, 4, N]
    wq_t = wq.rearrange("(c p) m -> p c m", p=128)          # [128, 4, 512]
    wk_t = wk.rearrange("(c p) m -> p c m", p=128)
    wv_t = wv.rearrange("(c p) m -> p c m", p=128)
    wo_t = wo.rearrange("(c p) m -> p c m", p=128)          # [128, 4, 512]
    eb_t = eb.rearrange("h (mc p) q -> h p mc q", p=128)    # [H, 128, 8, N]
    out_t = out.rearrange("b (nc p) d -> b p nc d", p=128)  # [B, 128, 8, 512]

    with tile.TileContext(nc) as tc:
        with (
            tc.tile_pool(name="consts", bufs=1) as consts,
            tc.tile_pool(name="xp", bufs=2) as xp,
            tc.tile_pool(name="acts", bufs=4) as acts,
            tc.tile_pool(name="qkp", bufs=9) as qkp,
            tc.tile_pool(name="ep", bufs=2) as ep,
            tc.tile_pool(name="pp", bufs=4) as pp,
            tc.tile_pool(name="attnp", bufs=3) as attnp,
            tc.tile_pool(name="aup", bufs=2) as aup,
            tc.tile_pool(name="lp", bufs=3) as lp,
            tc.tile_pool(name="outp", bufs=2) as outp,
            tc.tile_pool(name="ps_proj", bufs=2, space="PSUM") as ps_proj,
            tc.tile_pool(name="ps_s", bufs=2, space="PSUM") as ps_s,
            tc.tile_pool(name="ps_pv", bufs=1, space="PSUM") as ps_pv,
        ):
            wq_sb = consts.tile([128, DCHUNK, D], bf16, tag="wq")
            wk_sb = consts.tile([128, DCHUNK, D], bf16, tag="wk")
            wv_sb = consts.tile([128, DCHUNK, D], bf16, tag="wv")
            wo_sb = consts.tile([128, DCHUNK, D], bf16, tag="wo")
            bout_rep = consts.tile([128, D], f32, tag="boutr")
            bout_row = consts.tile([1, D], bf16, tag="boutrow")
            ones_sb = consts.tile([128, 1], bf16, tag="ones1")
            ones_row = consts.tile([1, 128], bf16, tag="onesrow")
            nc.gpsimd.memset(ones_row, 1.0)
            nc.gpsimd.memset(ones_sb, 1.0)

            # b -> {"qt": {pr: tile}, "kt": {pr: tile}, "vt": t, "attn2": t}
            tiles = {}
            xts = {}
            attnus = {}   # (b, pr) -> attnU tile
            eb_tiles = {}  # (bp, h) -> eb tile
            pvs = {}      # (b, h) -> pv psum tile

            def ensure_batch(b):
                if b not in xts:
                    xt_sb = xp.tile([128, DCHUNK, N], bf16, tag="xt", name=f"xt_{b}")
                    nc.sync.dma_start(xt_sb, xt_t[b])
                    xts[b] = xt_sb
                    tiles[b] = {"qt": {}, "kt": {}, "vt": None, "attn2": None}

            psq = {}  # open psum groups for split fillers

            def qk_group(b, pr, wi, half, part):
                # half a psum group: 2 matmuls (+ evac on the closing part)
                ensure_batch(b)
                if pr not in tiles[b]["qt"]:
                    tiles[b]["qt"][pr] = qkp.tile(
                        [128, N], bf16, tag="qt_pair", name=f"qt_{b}_{pr}"
                    )
                    tiles[b]["kt"][pr] = qkp.tile(
                        [128, N], bf16, tag="kt_pair", name=f"kt_{b}_{pr}"
                    )
                xt_sb = xts[b]
                w_sb = wq_sb if wi == 0 else wk_sb
                dst = tiles[b]["qt" if wi == 0 else "kt"][pr]
                key = ("qk", b, pr, wi, half)
                if part == 0:
                    psq[key] = ps_proj.tile([128, 512], f32, tag="ps_proj", name="psg")
                ps = psq[key]
                for dc in (0, 1) if part == 0 else (2, 3):
                    nc.tensor.matmul(
                        ps,
                        lhsT=w_sb[:, dc, pr * 128 : (pr + 1) * 128],
                        rhs=xt_sb[:, dc, half * 512 : (half + 1) * 512],
                        start=(dc == 0),
                        stop=(dc == DCHUNK - 1),
                    )
                if part == 1:
                    del psq[key]
                    nc.vector.tensor_copy(dst[:, half * 512 : (half + 1) * 512], ps)

            def qk_fillers(b, pr):
                return [
                    (
                        ("qk", b, pr, wi, half),
                        lambda wi=wi, half=half, part=part: qk_group(
                            b, pr, wi, half, part
                        ),
                    )
                    for wi in range(2)
                    for half in range(2)
                    for part in range(2)
                ]

            def v_group(b, nck, part):
                ensure_batch(b)
                if tiles[b]["vt"] is None:
                    tiles[b]["vt"] = acts.tile(
                        [128, NCHUNK, HEADS, DH + 1], bf16, tag="vt", name=f"vt_{b}"
                    )
                    nc.gpsimd.memset(tiles[b]["vt"][:, :, :, DH : DH + 1], 1.0)
                    tiles[b]["attn2"] = attnp.tile(
                        [128, DCHUNK, NCHUNK, 128], bf16, tag="attn2",
                        name=f"attn2_{b}",
                    )
                xt_sb = xts[b]
                vt_sb = tiles[b]["vt"]
                key = ("v", b, nck)
                if part == 0:
                    psq[key] = ps_proj.tile([128, 512], f32, tag="ps_proj", name="psg")
                ps = psq[key]
                for dc in (0, 1) if part == 0 else (2, 3):
                    nc.tensor.matmul(
                        ps,
                        lhsT=xt_sb[:, dc, nck * 128 : (nck + 1) * 128],
                        rhs=wv_sb[:, dc, :],
                        start=(dc == 0),
                        stop=(dc == DCHUNK - 1),
                    )
                if part == 1:
                    del psq[key]
                    nc.vector.tensor_copy(
                        vt_sb[:, nck, :, 0:DH],
                        ps.rearrange("p (h d) -> p h d", h=HEADS),
                    )

            def v_fillers(b, ncks):
                return [
                    (("v", b, nck), lambda nck=nck, part=part: v_group(b, nck, part))
                    for nck in ncks
                    for part in range(2)
                ]

            def proj_group(b, nck, part, store=4, on_act=False):
                attn2 = tiles[b]["attn2"]
                if b not in osbs:
                    osbs[b] = outp.tile(
                        [128, NCHUNK, D], bf16, tag="osb", name=f"osb_{b}"
                    )
                o_sb = osbs[b]
                key = ("proj", b, nck)
                if part == 0:
                    psq[key] = ps_proj.tile([128, 512], f32, tag="ps_proj", name="psg")
                ps = psq[key]
                for ic in (0, 1) if part == 0 else (2, 3):
                    nc.tensor.matmul(
                        ps,
                        lhsT=attn2[:, ic, nck, :],
                        rhs=wo_sb[:, ic, :],
                        start=(ic == 0),
                        stop=(ic == DCHUNK - 1) and not on_act,
                        skip_group_check=on_act and ic == DCHUNK - 1,
                    )
                if part == 1:
                    del psq[key]
                    if on_act:
                        # tail: ACT is idle; bias enters via a K=1 ones-row
                        # matmul so the evacuation is a plain ACT copy.
                        nc.tensor.matmul(
                            ps,
                            lhsT=ones_row,
                            rhs=bout_row,
                            start=False,
                            stop=True,
                            skip_group_check=True,
                        )
                        nc.scalar.activation(o_sb[:, nck, :], ps, AF.Copy)
                    else:
                        nc.vector.tensor_tensor(o_sb[:, nck, :], ps, bout_rep, OP.add)
                    if (nck + 1) % store == 0:
                        lo = nck + 1 - store
                        nc.sync.dma_start(
                            out_t[b][:, lo : nck + 1, :], o_sb[:, lo : nck + 1, :]
                        )

            def proj_fillers(b, ncks, store=4, on_act=False):
                return [
                    (
                        ("proj", b),
                        lambda nck=nck, part=part: proj_group(
                            b, nck, part, store, on_act
                        ),
                    )
                    for nck in ncks
                    for part in range(2)
                ]

            # ---- global PV lag queue + filler queue ----------------------
            pending_pv = []
            filler_q = []
            slot_i = [0]

            def pump(n=None):
                k = len(pending_pv) - PV_LAG if n is None else n
                for _ in range(max(k, 0)):
                    pending_pv.pop(0)()

            def slot_tick():
                slot_i[0] += 1
                pump()
                n = 2 if len(filler_q) >= 12 else (1 if filler_q else 0)
                for _ in range(min(n, len(filler_q))):
                    filler_q.pop(0)[1]()

            def emit_epilogue(b, h, on_act=False):
                pr, po = h // 2, (h % 2) * 64
                pv = pvs.pop((b, h))
                if h % 2 == 0:
                    attnus[(b, pr)] = aup.tile(
                        [128, NCHUNK, 128], bf16, tag="attnU", name=f"au_{b}_{pr}"
                    )
                attnU = attnus[(b, pr)]
                linv = lp.tile([128, NCHUNK], f32, tag="linv")
                pvv = pv.rearrange("p (qc e) -> p qc e", e=128)
                nc.vector.reciprocal_approx_fast(out=linv, in_=pvv[:, :, 64])
                for qc in range(NCHUNK):
                    if on_act:
                        # the tail's last head: ACT is idle by now, DVE is not
                        nc.scalar.activation(
                            attnU[:, qc, po : po + 64],
                            pv[:, qc * 128 : qc * 128 + 64],
                            AF.Copy,
                            scale=linv[:, qc : qc + 1],
                        )
                    else:
                        nc.vector.tensor_scalar(
                            attnU[:, qc, po : po + 64],
                            pv[:, qc * 128 : qc * 128 + 64],
                            linv[:, qc : qc + 1],
                            None,
                            op0=OP.mult,
                        )
                if h % 2 == 1:
                    attn2 = tiles[b]["attn2"]
                    nc.sync.dma_start_transpose(
                        attn2[:, pr, :, :], attnus.pop((b, pr))
                    )

            def make_pv(b, h, mc, p_t, act_epi=False):
                def closure():
                    drain_tags({("v", b, mc)})
                    vt_sb = tiles[b]["vt"]
                    if (b, h) not in pvs:
                        pvs[(b, h)] = ps_pv.tile(
                            [128, 1024], f32, tag="pv", name=f"pv_{b}_{h}"
                        )
                    pv = pvs[(b, h)]
                    first = mc == 0
                    last = mc == NCHUNK - 1
                    hb = NCHUNK // 2
                    for qc in range(NCHUNK):
                        bank_first = first and qc % hb == 0
                        bank_last = last and qc % hb == hb - 1
                        nc.tensor.matmul(
                            pv[:, qc * 128 : qc * 128 + 65],
                            lhsT=p_t[:, qc * 128 : (qc + 1) * 128],
                            rhs=vt_sb[:, mc, h, :],
                            start=bank_first,
                            stop=bank_last,
                            skip_group_check=not (bank_first or bank_last),
                        )
                    if last:
                        emit_epilogue(b, h, on_act=act_epi)

                return closure

            def load_eb2(bp, h):
                t = ep.tile([128, NCHUNK, N], bf16, tag="eb", name=f"eb_{bp}_{h}")
                hc = NCHUNK // 2
                nc.sync.dma_start(t[:, 0:hc, :], eb_t[h, :, 0:hc, :])
                nc.sync.dma_start(t[:, hc:, :], eb_t[h, :, hc:, :])
                eb_tiles[(bp, h)] = t

            def drain_tags(tags):
                keep = []
                for tag, c in filler_q:
                    if tag in tags:
                        c()
                    else:
                        keep.append((tag, c))
                filler_q[:] = keep

            def emit_head(bp, bpair, h, junction=None):
                po = (h % 2) * 64
                pr = h // 2
                if h + 1 < HEADS:
                    load_eb2(bp, h + 1)
                elif bp + 1 < B_LOC // 2:
                    load_eb2(bp + 1, 0)
                eb_sb = eb_tiles.pop((bp, h))

                border = bpair if not (bp == B_LOC // 2 - 1 and h >= 6) else bpair[::-1]
                for b in border:
                    if b == border[1] and junction is not None:
                        junction()
                    # correctness backstop: producers this section reads must
                    # already be emitted (deps follow emission order; a
                    # later-emitted producer would race on HW). Usually a
                    # no-op -- the push plan leads by >= 2 heads.
                    drain_tags({
                        ("qk", b, pr, 0, 0),
                        ("qk", b, pr, 0, 1),
                        ("qk", b, pr, 1, 0),
                        ("qk", b, pr, 1, 1),
                    })
                    qt, kt = tiles[b]["qt"][pr], tiles[b]["kt"][pr]
                    for mc in range(NCHUNK):
                        ps = ps_s.tile([128, N], f32, tag="ps_s")
                        for half in range(2):
                            nc.tensor.matmul(
                                ps[:, half * 512 : (half + 1) * 512],
                                lhsT=kt[po : po + 64, mc * 128 : (mc + 1) * 128],
                                rhs=qt[po : po + 64, half * 512 : (half + 1) * 512],
                                start=True,
                                stop=True,
                            )
                        p_raw = pp.tile([128, N], bf16, tag="praw")
                        nc.scalar.activation(p_raw, ps, AF.Exp)
                        p_t = pp.tile([128, N], bf16, tag="pt", bufs=PV_LAG + 2)
                        pset = POOL_MC if h < HEADS - 1 else POOL_MC[:2]
                        eng = nc.gpsimd if mc in pset else nc.vector
                        eng.tensor_tensor(p_t, p_raw, eb_sb[:, mc, :], OP.mult)
                        act_epi = bp == B_LOC // 2 - 1 and h == HEADS - 1 and b == border[-1]
                        pending_pv.append(make_pv(b, h, mc, p_t, act_epi))
                        slot_tick()
                    # drain the lag queue at the section end so the epilogue
                    # (and the PV psum release) is emitted ~4 slots earlier.
                    pump(len(pending_pv) - 4)

            osbs = {}
            H1 = list(range(0, NCHUNK // 2))
            H2 = list(range(NCHUNK // 2, NCHUNK))

            # bootstrap: xt0 + Q/K weights lead the DMA queue, then the
            # first eb; V(0) streams in as fillers during head 0. A dummy
            # matmul stream keeps the PE busy through the DMA window so the
            # p-state is warm when the real projections start.
            nc.sync.dma_start(wq_sb, wq_t)
            ensure_batch(0)
            nc.sync.dma_start(wk_sb, wk_t)
            nc.sync.dma_start(wv_sb, wv_t)
            load_eb2(0, 0)
            ensure_batch(1)
            nc.sync.dma_start(wo_sb, wo_t)
            nc.sync.dma_start(bout_rep, bout[:, :].to_broadcast((128, D)))
            nc.vector.tensor_copy(bout_row, bout_rep[0:1, :])
            warm_src = consts.tile([1, 512], bf16, tag="warmsrc")
            nc.gpsimd.memset(warm_src, 0.0)
            ps_warm = ps_proj.tile([1, 512], f32, tag="ps_proj", name="ps_warm")
            for i in range(14):
                nc.tensor.matmul(
                    ps_warm,
                    lhsT=ones_sb[0:1, 0:1],
                    rhs=warm_src,
                    start=(i == 0),
                    stop=(i == 13),
                )
            for _, c in qk_fillers(0, 0):
                c()
            filler_q.extend(v_fillers(0, list(range(NCHUNK))))

            def j_boot():
                # batch 1's Q/K pair 0 must be in the PE stream before its
                # S matmuls; its V chunks stream as fillers (the backstop in
                # emit_head force-drains them before b1's first slot).
                for _, c in qk_fillers(1, 0):
                    c()
                filler_q.extend(v_fillers(1, list(range(NCHUNK))))

            pending_proj = []
            for bp in range(B_LOC // 2):
                bpair = (2 * bp, 2 * bp + 1)
                nxt = (2 * bp + 2, 2 * bp + 3) if bp + 1 < B_LOC // 2 else None
                for h in range(HEADS):
                    # push plan: <=4 filler groups per half-head
                    pushes_a, pushes_b = [], []
                    if h < 6:
                        tb = bpair[h % 2]
                        tpr = h // 2 + 1
                        if tpr < 4:
                            pushes_a = qk_fillers(tb, tpr)
                    if h == 2 and pending_proj:
                        pushes_b = proj_fillers(pending_proj[0], H1)
                    elif h == 3 and pending_proj:
                        pushes_b = proj_fillers(pending_proj.pop(0), H2)
                    elif h == 4 and pending_proj:
                        pushes_b = proj_fillers(pending_proj[0], H1)
                    elif h == 5 and pending_proj:
                        pushes_b = proj_fillers(pending_proj.pop(0), H2)
                    if nxt is not None:
                        if h == 5:
                            pushes_a = pushes_a + qk_fillers(nxt[0], 0)
                            pushes_b = v_fillers(nxt[0], H1)
                        elif h == 6:
                            pushes_a = v_fillers(nxt[0], H2)
                            pushes_b = qk_fillers(nxt[1], 0)
                        elif h == 7:
                            pushes_a = v_fillers(nxt[1], H1)
                            pushes_b = v_fillers(nxt[1], H2)

                    filler_q.extend(pushes_a)

                    def junction(pb=pushes_b, bp0=bp, h0=h):
                        if bp0 == 0 and h0 == 0:
                            j_boot()
                        filler_q.extend(pb)

                    emit_head(bp, bpair, h, junction=junction)
                pending_proj += [bpair[0], bpair[1]]

            # tail: drain fillers, then interleave the last pair's output
            # projections with the remaining PV pops.
            while filler_q:
                filler_q.pop(0)[1]()
            b2, b3 = pending_proj
            pump(len(pending_pv))
            for _, c in proj_fillers(b3, H1) + proj_fillers(b3, H2):
                c()
            for _, c in proj_fillers(b2, H1) + proj_fillers(b2, H2, store=2):
                c()

    nc.compile()
    return nc, "out"


def get_program():
    global _PROG
    if _PROG is None:
        _PROG = _build_program()
    return _PROG


def shard_inputs(x, w_qkv, w_out, b_out, bias_table, rel_index):
    bf = ml_dtypes.bfloat16
    x = np.asarray(x, np.float32)
    w_qkv = np.asarray(w_qkv, np.float32)
    w_out = np.asarray(w_out, np.float32)
    b_out = np.asarray(b_out, np.float32)
    bias_table = np.asarray(bias_table, np.float32)
    rel_index = np.asarray(rel_index)

    scale = DH ** -0.5
    wq = np.ascontiguousarray(w_qkv[:, 0:D] * scale).astype(bf)
    wk = np.ascontiguousarray(w_qkv[:, D : 2 * D]).astype(bf)
    wv = np.ascontiguousarray(w_qkv[:, 2 * D : 3 * D]).astype(bf)
    wo = np.ascontiguousarray(w_out).astype(bf)
    bout = np.ascontiguousarray(b_out[None, :]).astype(np.float32)
    # exp(bias)^T per head: eb[h, m, q] = exp(bias_table[rel_index[q, m], h])
    biasT = bias_table[rel_index].transpose(2, 1, 0)  # [H, m, q]
    ebv = np.ascontiguousarray(np.exp(biasT).astype(bf))

    in_maps = []
    for c in range(N_CORES):
        xs = x[c * B_LOC : (c + 1) * B_LOC]  # [4, 1024, 512]
        xts = np.ascontiguousarray(xs.transpose(0, 2, 1)).astype(bf)
        in_maps.append(
            {
                "xt": xts,
                "wq": wq,
                "wk": wk,
                "wv": wv,
                "wo": wo,
                "bout": bout,
                "eb": ebv,
            }
        )
    return in_maps


def kernel(x, w_qkv, w_out, b_out, bias_table, rel_index):
    from concourse.bass_utils import run_bass_kernel_spmd

    nc, out_name = get_program()
    in_maps = shard_inputs(x, w_qkv, w_out, b_out, bias_table, rel_index)
    try:
        res = run_bass_kernel_spmd(nc, in_maps, core_ids=list(range(N_CORES)))
    except Exception:
        # transient device errors (e.g. NRT_EXEC_UNIT_UNRECOVERABLE) have been
        # observed once on an otherwise-passing kernel; retry once
        res = run_bass_kernel_spmd(nc, in_maps, core_ids=list(range(N_CORES)))
    outs = [r[out_name] for r in res.results]
    return np.concatenate(outs, axis=0).astype(np.float32)


# revision 50
# speedup vs baseline: 1.0001x; 1.0001x over previous
"""Trainium2 Bass kernel for nn_AttentionRelative (Swin-style relative-position-bias MHA).

Full-problem shapes: x [32, 1024, 512], HEADS=8, DIM_HEAD=64.
Sharding: data-parallel over batch across 8 NeuronCores (4 batches/core);
weights and the (host-gathered, exp'd) bias table replicated.

Device algorithm per (batch, head), all matmul inputs bf16:
  - qkv projection from host-pre-transposed xT, producing qT/kT in
    [head_dim on partitions, n free] layout (one tile per head-pair,
    allocated lazily) and V in [n on partitions, (head, head_dim) free]
    layout.
  - S^T = K @ Q^T (keys m on partitions, queries q free) -> PSUM.
  - P^T = exp(S^T) * exp(biasT)  -- ACT does exp straight out of PSUM,
    the multiply by the host-precomputed exp(bias) runs on DVE (bf16 2x)
    for six of eight chunks and on GPSIMD for two (engine balance).
  - PV is computed TRANSPOSED: out[q, d] accumulated per q-chunk with
    lhsT = P^T chunk (stationary), rhs = V chunk -- free size 64 per
    matmul instead of 512, plus per-q-chunk l columns via a ones vector
    (softmax denominator lands per-partition, no DRAM broadcast bounce).
    PSUM bank-clear semantics: one accumulation group per bank (start on
    the first matmul touching the bank, stop on the last).
  - The PV+l emission lags the S/exp/mult chain by PV_LAG mc-slots
    (global software pipeline) so the in-order PE stream never stalls
    waiting for the ACT->DVE/GPSIMD latency chain.
  - epilogue (emitted when a head's last PV pops off the lag queue):
    fast reciprocal of the l columns, then 8 per-partition tensor_scalar
    multiplies evacuate+normalize PV psum into attnU [q, qc, dpair]
    bf16; after both heads of a pair, one DMA-transpose turns attnU
    into the attn2 [dpair, qc, q] lhsT layout the output projection
    needs (14ns per 32x32 tile on the DMA engines).
  - out-proj accumulates over inner chunks; b_out is added during the
    psum evacuation (tensor_tensor add with a broadcast bias tile), and
    the per-batch result is stored with a single DMA in bf16.
  - batches are processed in pairs sharing one bias-table load per head
    (one large DMA per (pair, head), prefetched a head ahead); the next
    pair's projections are emitted in small pieces at head boundaries so
    the scheduler places them into the attention phase's engine bubbles.
"""

import numpy as np
import ml_dtypes

B_FULL = 32
N_CORES = 8
B_LOC = B_FULL // N_CORES  # 4
N = 1024
D = 512
HEADS = 8
DH = 64
NCHUNK = N // 128  # 8
DCHUNK = D // 128  # 4
PV_LAG = 6  # mc-slots of lag between the S/exp/mult chain and PV emission
POOL_MC = (0, 4, 6)  # mc chunks whose exp(bias) multiply runs on GPSIMD

_PROG = None  # (nc, out_name) built once per process


def _build_program():
    import concourse.mybir as mybir
    import concourse.tile as tile
    from concourse import bacc

    f32 = mybir.dt.float32
    bf16 = mybir.dt.bfloat16
    AF = mybir.ActivationFunctionType
    OP = mybir.AluOpType

    nc = bacc.Bacc(target_bir_lowering=False)

    xt = nc.dram_tensor("xt", [B_LOC, D, N], bf16, kind="ExternalInput")
    wq = nc.dram_tensor("wq", [D, D], bf16, kind="ExternalInput")
    wk = nc.dram_tensor("wk", [D, D], bf16, kind="ExternalInput")
    wv = nc.dram_tensor("wv", [D, D], bf16, kind="ExternalInput")
    wo = nc.dram_tensor("wo", [D, D], bf16, kind="ExternalInput")
    bout = nc.dram_tensor("bout", [1, D], f32, kind="ExternalInput")
    eb = nc.dram_tensor("eb", [HEADS, N, N], bf16, kind="ExternalInput")
    out = nc.dram_tensor("out", [B_LOC, N, D], bf16, kind="ExternalOutput")

    xt_t = xt.rearrange("b (c p) n -> b p c n", p=128)      # [B, 128, 4, N]
    wq_t = wq.rearrange("(c p) m -> p c m", p=128)          # [128, 4, 512]
    wk_t = wk.rearrange("(c p) m -> p c m", p=128)
    wv_t = wv.rearrange("(c p) m -> p c m", p=128)
    wo_t = wo.rearrange("(c p) m -> p c m", p=128)          # [128, 4, 512]
    eb_t = eb.rearrange("h (mc p) q -> h p mc q", p=128)    # [H, 128, 8, N]
    out_t = out.rearrange("b (nc p) d -> b p nc d", p=128)  # [B, 128, 8, 512]

    with tile.TileContext(nc) as tc:
        with (
            tc.tile_pool(name="consts", bufs=1) as consts,
            tc.tile_pool(name="xp", bufs=2) as xp,
            tc.tile_pool(name="acts", bufs=4) as acts,
            tc.tile_pool(name="qkp", bufs=9) as qkp,
            tc.tile_pool(name="ep", bufs=2) as ep,
            tc.tile_pool(name="pp", bufs=4) as pp,
            tc.tile_pool(name="attnp", bufs=3) as attnp,
            tc.tile_pool(name="aup", bufs=2) as aup,
            tc.tile_pool(name="lp", bufs=3) as lp,
            tc.tile_pool(name="outp", bufs=2) as outp,
            tc.tile_pool(name="ps_proj", bufs=2, space="PSUM") as ps_proj,
            tc.tile_pool(name="ps_s", bufs=2, space="PSUM") as ps_s,
            tc.tile_pool(name="ps_pv", bufs=1, space="PSUM") as ps_pv,
        ):
            wq_sb = consts.tile([128, DCHUNK, D], bf16, tag="wq")
            wk_sb = consts.tile([128, DCHUNK, D], bf16, tag="wk")
            wv_sb = consts.tile([128, DCHUNK, D], bf16, tag="wv")
            wo_sb = consts.tile([128, DCHUNK, D], bf16, tag="wo")
            bout_rep = consts.tile([128, D], f32, tag="boutr")
            bout_row = consts.tile([1, D], bf16, tag="boutrow")
            ones_sb = consts.tile([128, 1], bf16, tag="ones1")
            ones_row = consts.tile([1, 128], bf16, tag="onesrow")
            nc.gpsimd.memset(ones_row, 1.0)
            nc.gpsimd.memset(ones_sb, 1.0)

            # b -> {"qt": {pr: tile}, "kt": {pr: tile}, "vt": t, "attn2": t}
            tiles = {}
            xts = {}
            attnus = {}   # (b, pr) -> attnU tile
            eb_tiles = {}  # (bp, h) -> eb tile
            pvs = {}      # (b, h) -> pv psum tile

            def ensure_batch(b):
                if b not in xts:
                    xt_sb = xp.tile([128, DCHUNK, N], bf16, tag="xt", name=f"xt_{b}")
                    nc.sync.dma_start(xt_sb, xt_t[b])
                    xts[b] = xt_sb
                    tiles[b] = {"qt": {}, "kt": {}, "vt": None, "attn2": None}

            psq = {}  # open psum groups for split fillers

            def qk_group(b, pr, wi, half, part):
                # half a psum group: 2 matmuls (+ evac on the closing part)
                ensure_batch(b)
                if pr not in tiles[b]["qt"]:
                    tiles[b]["qt"][pr] = qkp.tile(
                        [128, N], bf16, tag="qt_pair", name=f"qt_{b}_{pr}"
                    )
                    tiles[b]["kt"][pr] = qkp.tile(
                        [128, N], bf16, tag="kt_pair", name=f"kt_{b}_{pr}"
                    )
                xt_sb = xts[b]
                w_sb = wq_sb if wi == 0 else wk_sb
                dst = tiles[b]["qt" if wi == 0 else "kt"][pr]
                key = ("qk", b, pr, wi, half)
                if part == 0:
                    psq[key] = ps_proj.tile([128, 512], f32, tag="ps_proj", name="psg")
                ps = psq[key]
                for dc in (0, 1) if part == 0 else (2, 3):
                    nc.tensor.matmul(
                        ps,
                        lhsT=w_sb[:, dc, pr * 128 : (pr + 1) * 128],
                        rhs=xt_sb[:, dc, half * 512 : (half + 1) * 512],
                        start=(dc == 0),
                        stop=(dc == DCHUNK - 1),
                    )
                if part == 1:
                    del psq[key]
                    nc.vector.tensor_copy(dst[:, half * 512 : (half + 1) * 512], ps)

            def qk_fillers(b, pr):
                return [
                    (
                        ("qk", b, pr, wi, half),
                        lambda wi=wi, half=half, part=part: qk_group(
                            b, pr, wi, half, part
                        ),
                    )
                    for wi in range(2)
                    for half in range(2)
                    for part in range(2)
                ]

            def v_group(b, nck, part):
                ensure_batch(b)
                if tiles[b]["vt"] is None:
                    tiles[b]["vt"] = acts.tile(
                        [128, NCHUNK, HEADS, DH + 1], bf16, tag="vt", name=f"vt_{b}"
                    )
                    nc.gpsimd.memset(tiles[b]["vt"][:, :, :, DH : DH + 1], 1.0)
                    tiles[b]["attn2"] = attnp.tile(
                        [128, DCHUNK, NCHUNK, 128], bf16, tag="attn2",
                        name=f"attn2_{b}",
                    )
                xt_sb = xts[b]
                vt_sb = tiles[b]["vt"]
                key = ("v", b, nck)
                if part == 0:
                    psq[key] = ps_proj.tile([128, 512], f32, tag="ps_proj", name="psg")
                ps = psq[key]
                for dc in (0, 1) if part == 0 else (2, 3):
                    nc.tensor.matmul(
                        ps,
                        lhsT=xt_sb[:, dc, nck * 128 : (nck + 1) * 128],
                        rhs=wv_sb[:, dc, :],
                        start=(dc == 0),
                        stop=(dc == DCHUNK - 1),
                    )
                if part == 1:
                    del psq[key]
                    nc.vector.tensor_copy(
                        vt_sb[:, nck, :, 0:DH],
                        ps.rearrange("p (h d) -> p h d", h=HEADS),
                    )

            def v_fillers(b, ncks):
                return [
                    (("v", b, nck), lambda nck=nck, part=part: v_group(b, nck, part))
                    for nck in ncks
                    for part in range(2)
                ]

            def proj_group(b, nck, part, store=4, on_act=False):
                attn2 = tiles[b]["attn2"]
                if b not in osbs:
                    osbs[b] = outp.tile(
                        [128, NCHUNK, D], bf16, tag="osb", name=f"osb_{b}"
                    )
                o_sb = osbs[b]
                key = ("proj", b, nck)
                if part == 0:
                    psq[key] = ps_proj.tile([128, 512], f32, tag="ps_proj", name="psg")
                ps = psq[key]
                for ic in (0, 1) if part == 0 else (2, 3):
                    nc.tensor.matmul(
                        ps,
                        lhsT=attn2[:, ic, nck, :],
                        rhs=wo_sb[:, ic, :],
                        start=(ic == 0),
                        stop=(ic == DCHUNK - 1) and not on_act,
                        skip_group_check=on_act and ic == DCHUNK - 1,
                    )
                if part == 1:
                    del psq[key]
                    if on_act:
                        # tail: ACT is idle; bias enters via a K=1 ones-row
                        # matmul so the evacuation is a plain ACT copy.
                        nc.tensor.matmul(
                            ps,
                            lhsT=ones_row,
                            rhs=bout_row,
                            start=False,
                            stop=True,
                            skip_group_check=True,
                        )
                        nc.scalar.activation(o_sb[:, nck, :], ps, AF.Copy)
                    else:
                        nc.vector.tensor_tensor(o_sb[:, nck, :], ps, bout_rep, OP.add)
                    if (nck + 1) % store == 0:
                        lo = nck + 1 - store
                        nc.sync.dma_start(
                            out_t[b][:, lo : nck + 1, :], o_sb[:, lo : nck + 1, :]
                        )

            def proj_fillers(b, ncks, store=4, on_act=False):
                return [
                    (
                        ("proj", b),
                        lambda nck=nck, part=part: proj_group(
                            b, nck, part, store, on_act
                        ),
                    )
                    for nck in ncks
                    for part in range(2)
                ]

            # ---- global PV lag queue + filler queue ----------------------
            pending_pv = []
            filler_q = []
            slot_i = [0]

            def pump(n=None):
                k = len(pending_pv) - PV_LAG if n is None else n
                for _ in range(max(k, 0)):
                    pending_pv.pop(0)()

            def slot_tick():
                slot_i[0] += 1
                pump()
                n = 2 if len(filler_q) >= 12 else (1 if filler_q else 0)
                for _ in range(min(n, len(filler_q))):
                    filler_q.pop(0)[1]()

            def emit_epilogue(b, h, on_act=False):
                pr, po = h // 2, (h % 2) * 64
                pv = pvs.pop((b, h))
                if h % 2 == 0:
                    attnus[(b, pr)] = aup.tile(
                        [128, NCHUNK, 128], bf16, tag="attnU", name=f"au_{b}_{pr}"
                    )
                attnU = attnus[(b, pr)]
                linv = lp.tile([128, NCHUNK], f32, tag="linv")
                pvv = pv.rearrange("p (qc e) -> p qc e", e=128)
                nc.vector.reciprocal_approx_fast(out=linv, in_=pvv[:, :, 64])
                for qc in range(NCHUNK):
                    if on_act:
                        # the tail's last head: ACT is idle by now, DVE is not
                        nc.scalar.activation(
                            attnU[:, qc, po : po + 64],
                            pv[:, qc * 128 : qc * 128 + 64],
                            AF.Copy,
                            scale=linv[:, qc : qc + 1],
                        )
                    else:
                        nc.vector.tensor_scalar(
                            attnU[:, qc, po : po + 64],
                            pv[:, qc * 128 : qc * 128 + 64],
                            linv[:, qc : qc + 1],
                            None,
                            op0=OP.mult,
                        )
                if h % 2 == 1:
                    attn2 = tiles[b]["attn2"]
                    nc.sync.dma_start_transpose(
                        attn2[:, pr, :, :], attnus.pop((b, pr))
                    )

            def make_pv(b, h, mc, p_t, act_epi=False):
                def closure():
                    drain_tags({("v", b, mc)})
                    vt_sb = tiles[b]["vt"]
                    if (b, h) not in pvs:
                        pvs[(b, h)] = ps_pv.tile(
                            [128, 1024], f32, tag="pv", name=f"pv_{b}_{h}"
                        )
                    pv = pvs[(b, h)]
                    first = mc == 0
                    last = mc == NCHUNK - 1
                    hb = NCHUNK // 2
                    for qc in range(NCHUNK):
                        bank_first = first and qc % hb == 0
                        bank_last = last and qc % hb == hb - 1
                        nc.tensor.matmul(
                            pv[:, qc * 128 : qc * 128 + 65],
                            lhsT=p_t[:, qc * 128 : (qc + 1) * 128],
                            rhs=vt_sb[:, mc, h, :],
                            start=bank_first,
                            stop=bank_last,
                            skip_group_check=not (bank_first or bank_last),
                        )
                    if last:
                        emit_epilogue(b, h, on_act=act_epi)

                return closure

            def load_eb2(bp, h):
                t = ep.tile([128, NCHUNK, N], bf16, tag="eb", name=f"eb_{bp}_{h}")
                hc = NCHUNK // 2
                nc.sync.dma_start(t[:, 0:hc, :], eb_t[h, :, 0:hc, :])
                nc.sync.dma_start(t[:, hc:, :], eb_t[h, :, hc:, :])
                eb_tiles[(bp, h)] = t

            def drain_tags(tags):
                keep = []
                for tag, c in filler_q:
                    if tag in tags:
                        c()
                    else:
                        keep.append((tag, c))
                filler_q[:] = keep

            def emit_head(bp, bpair, h, junction=None):
                po = (h % 2) * 64
                pr = h // 2
                if h + 1 < HEADS:
                    load_eb2(bp, h + 1)
                elif bp + 1 < B_LOC // 2:
                    load_eb2(bp + 1, 0)
                eb_sb = eb_tiles.pop((bp, h))

                border = bpair if not (bp == B_LOC // 2 - 1 and h >= 6) else bpair[::-1]
                for b in border:
                    if b == border[1] and junction is not None:
                        junction()
                    # correctness backstop: producers this section reads must
                    # already be emitted (deps follow emission order; a
                    # later-emitted producer would race on HW). Usually a
                    # no-op -- the push plan leads by >= 2 heads. K half1 is
                    # only read from mc=4 on, so it may stream in as a filler.
                    drain_tags({
                        ("qk", b, pr, 0, 0),
                        ("qk", b, pr, 0, 1),
                        ("qk", b, pr, 1, 0),
                    })
                    qt, kt = tiles[b]["qt"][pr], tiles[b]["kt"][pr]
                    for mc in range(NCHUNK):
                        if mc == NCHUNK // 2:
                            drain_tags({("qk", b, pr, 1, 1)})
                        ps = ps_s.tile([128, N], f32, tag="ps_s")
                        for half in range(2):
                            nc.tensor.matmul(
                                ps[:, half * 512 : (half + 1) * 512],
                                lhsT=kt[po : po + 64, mc * 128 : (mc + 1) * 128],
                                rhs=qt[po : po + 64, half * 512 : (half + 1) * 512],
                                start=True,
                                stop=True,
                            )
                        p_raw = pp.tile([128, N], bf16, tag="praw")
                        nc.scalar.activation(p_raw, ps, AF.Exp)
                        p_t = pp.tile([128, N], bf16, tag="pt", bufs=PV_LAG + 2)
                        pset = POOL_MC if h < HEADS - 1 else POOL_MC[:2]
                        eng = nc.gpsimd if mc in pset else nc.vector
                        eng.tensor_tensor(p_t, p_raw, eb_sb[:, mc, :], OP.mult)
                        act_epi = bp == B_LOC // 2 - 1 and h == HEADS - 1 and b == border[-1]
                        pending_pv.append(make_pv(b, h, mc, p_t, act_epi))
                        slot_tick()
                    # drain the lag queue at the section end so the epilogue
                    # (and the PV psum release) is emitted ~4 slots earlier.
                    pump(len(pending_pv) - 4)

            osbs = {}
            H1 = list(range(0, NCHUNK // 2))
            H2 = list(range(NCHUNK // 2, NCHUNK))

            # bootstrap: xt0 + Q/K weights lead the DMA queue, then the
            # first eb; V(0) streams in as fillers during head 0. A dummy
            # matmul stream keeps the PE busy through the DMA window so the
            # p-state is warm when the real projections start.
            nc.sync.dma_start(wq_sb, wq_t)
            ensure_batch(0)
            nc.sync.dma_start(wk_sb, wk_t)
            nc.sync.dma_start(wv_sb, wv_t)
            load_eb2(0, 0)
            ensure_batch(1)
            nc.sync.dma_start(wo_sb, wo_t)
            nc.sync.dma_start(bout_rep, bout[:, :].to_broadcast((128, D)))
            nc.vector.tensor_copy(bout_row, bout_rep[0:1, :])
            warm_src = consts.tile([1, 512], bf16, tag="warmsrc")
            nc.gpsimd.memset(warm_src, 0.0)
            ps_warm = ps_proj.tile([1, 512], f32, tag="ps_proj", name="ps_warm")
            for i in range(14):
                nc.tensor.matmul(
                    ps_warm,
                    lhsT=ones_sb[0:1, 0:1],
                    rhs=warm_src,
                    start=(i == 0),
                    stop=(i == 13),
                )
            for _, c in qk_fillers(0, 0):
                c()
            filler_q.extend(v_fillers(0, list(range(NCHUNK))))

            def j_boot():
                # batch 1's Q/K pair 0 (minus K half1, first read at mc=4)
                # must be in the PE stream before its S matmuls; the rest
                # streams as fillers guarded by the emit_head backstops.
                front = []
                for tag, c in qk_fillers(1, 0):
                    if tag[3:] == (1, 1):
                        front.append((tag, c))
                    else:
                        c()
                filler_q[:0] = front
                filler_q.extend(v_fillers(1, list(range(NCHUNK))))

            pending_proj = []
            for bp in range(B_LOC // 2):
                bpair = (2 * bp, 2 * bp + 1)
                nxt = (2 * bp + 2, 2 * bp + 3) if bp + 1 < B_LOC // 2 else None
                for h in range(HEADS):
                    # push plan: <=4 filler groups per half-head
                    pushes_a, pushes_b = [], []
                    if h < 6:
                        tb = bpair[h % 2]
                        tpr = h // 2 + 1
                        if tpr < 4:
                            pushes_a = qk_fillers(tb, tpr)
                    if h == 2 and pending_proj:
                        pushes_b = proj_fillers(pending_proj[0], H1)
                    elif h == 3 and pending_proj:
                        pushes_b = proj_fillers(pending_proj.pop(0), H2)
                    elif h == 4 and pending_proj:
                        pushes_b = proj_fillers(pending_proj[0], H1)
                    elif h == 5 and pending_proj:
                        pushes_b = proj_fillers(pending_proj.pop(0), H2)
                    if nxt is not None:
                        if h == 5:
                            pushes_a = pushes_a + qk_fillers(nxt[0], 0)
                            pushes_b = v_fillers(nxt[0], H1)
                        elif h == 6:
                            pushes_a = v_fillers(nxt[0], H2)
                            pushes_b = qk_fillers(nxt[1], 0)
                        elif h == 7:
                            pushes_a = v_fillers(nxt[1], H1)
                            pushes_b = v_fillers(nxt[1], H2)

                    filler_q.extend(pushes_a)

                    def junction(pb=pushes_b, bp0=bp, h0=h):
                        if bp0 == 0 and h0 == 0:
                            j_boot()
                        filler_q.extend(pb)

                    emit_head(bp, bpair, h, junction=junction)
                pending_proj += [bpair[0], bpair[1]]

            # tail: drain fillers, then interleave the last pair's output
            # projections with the remaining PV pops.
            while filler_q:
                filler_q.pop(0)[1]()
            b2, b3 = pending_proj
            pump(len(pending_pv))
            for _, c in proj_fillers(b3, H1) + proj_fillers(b3, H2):
                c()
            for _, c in proj_fillers(b2, H1) + proj_fillers(b2, H2, store=2):
                c()

    nc.compile()
    return nc, "out"


def get_program():
    global _PROG
    if _PROG is None:
        _PROG = _build_program()
    return _PROG


def shard_inputs(x, w_qkv, w_out, b_out, bias_table, rel_index):
    bf = ml_dtypes.bfloat16
    x = np.asarray(x, np.float32)
    w_qkv = np.asarray(w_qkv, np.float32)
    w_out = np.asarray(w_out, np.float32)
    b_out = np.asarray(b_out, np.float32)
    bias_table = np.asarray(bias_table, np.float32)
    rel_index = np.asarray(rel_index)

    scale = DH ** -0.5
    wq = np.ascontiguousarray(w_qkv[:, 0:D] * scale).astype(bf)
    wk = np.ascontiguousarray(w_qkv[:, D : 2 * D]).astype(bf)
    wv = np.ascontiguousarray(w_qkv[:, 2 * D : 3 * D]).astype(bf)
    wo = np.ascontiguousarray(w_out).astype(bf)
    bout = np.ascontiguousarray(b_out[None, :]).astype(np.float32)
    # exp(bias)^T per head: eb[h, m, q] = exp(bias_table[rel_index[q, m], h])
    biasT = bias_table[rel_index].transpose(2, 1, 0)  # [H, m, q]
    ebv = np.ascontiguousarray(np.exp(biasT).astype(bf))

    in_maps = []
    for c in range(N_CORES):
        xs = x[c * B_LOC : (c + 1) * B_LOC]  # [4, 1024, 512]
        xts = np.ascontiguousarray(xs.transpose(0, 2, 1)).astype(bf)
        in_maps.append(
            {
                "xt": xts,
                "wq": wq,
                "wk": wk,
                "wv": wv,
                "wo": wo,
                "bout": bout,
                "eb": ebv,
            }
        )
    return in_maps


def kernel(x, w_qkv, w_out, b_out, bias_table, rel_index):
    from concourse.bass_utils import run_bass_kernel_spmd

    nc, out_name = get_program()
    in_maps = shard_inputs(x, w_qkv, w_out, b_out, bias_table, rel_index)
    try:
        res = run_bass_kernel_spmd(nc, in_maps, core_ids=list(range(N_CORES)))
    except Exception:
        # transient device errors (e.g. NRT_EXEC_UNIT_UNRECOVERABLE) have been
        # observed once on an otherwise-passing kernel; retry once
        res = run_bass_kernel_spmd(nc, in_maps, core_ids=list(range(N_CORES)))
    outs = [r[out_name] for r in res.results]
    return np.concatenate(outs, axis=0).astype(np.float32)
